# revision 24
# baseline (speedup 1.0000x reference)
"""Trainium2 Bass kernel for nn_CWT_54872502174093.

CWT of a batch of signals with a 64-scale mexican-hat filter bank.

Math: the reference computes, per scale s (1..64),
    coef[b,s,n] = -sqrt(s) * diff(conv_full(x[b], K_s))[start_s + n]
which is algebraically a direct correlation
    coef[b,s,n] = sum_k g_s[k] * x[b, n + k - SH_s]
with g_s = reversed(-sqrt(s) * diff-extended(K_s)) (16s+2 taps) and
SH_s = len(K_s) - start_s - 1 = 8s+1.

On-chip this is evaluated as banded-Toeplitz matmuls in float32r
(1 cycle/row for moving free >= 256 on TRN2):
  stationary = transposed signal chunk [128 pos, (2 n-halves x 64 batch)]
  moving     = host-precomputed skewed filter matrix slice [128, 256]
  PSUM accumulates over tap chunks m.
Sharding: data-parallel over batch, 64 batches per core, 8 cores.
"""

import numpy as np

SIG = 2048
NS = 64
NCORES = 8
BC = 64          # batch per core
NCHUNK = 26      # position chunks of 128 in the padded transposed signal
LPAD = 5         # left zero-pad chunks (640 positions)
J = 256          # output tile width (moving free size)
NQ = 4           # q groups: n0 = 256*q
NH = 2           # n halves packed into stationary columns (offset 1024)
WMAX = 1536      # max filter-matrix width (scale 64)

_CACHE = {}


def _build_filters(kernels, scales_sqrt, trim_idx):
    """Per-scale skewed filter matrices W_s[p, i] = g_s[i0 + p - i], packed
    side by side into one [128, totcols] f32 array. Returns (Wall, specs)
    with specs[s-1] = (coloff, m_hi, c)."""
    kernels = np.asarray(kernels, np.float64)
    scales_sqrt = np.asarray(scales_sqrt, np.float64)
    trim_idx = np.asarray(trim_idx)
    cols = []
    specs = []
    totcols = 0
    for s in range(1, NS + 1):
        L = 16 * s + 1
        Karr = np.zeros(L + 2, np.float64)
        Karr[1:L + 1] = kernels[s - 1, :L]
        Wl = -scales_sqrt[s - 1] * (Karr[1:] - Karr[:-1])  # len L+1
        g = Wl[::-1].copy()
        Ks = L + 1
        start = int(trim_idx[s - 1, 0])
        SH = L - start - 1
        c = -(-SH // 128)
        m_hi = (Ks + 254 - SH) // 128 + c
        i0 = 128 * (m_hi - c) + SH
        Wd = 128 * m_hi + J
        P = np.arange(128)[:, None]
        I = np.arange(Wd)[None, :]
        gi = i0 + P - I
        Wbuf = np.where((gi >= 0) & (gi < Ks), g[np.clip(gi, 0, Ks - 1)], 0.0)
        cols.append(Wbuf.astype(np.float32))
        specs.append((totcols, m_hi, c))
        totcols += Wd
    return np.ascontiguousarray(np.concatenate(cols, axis=1)), specs


def _build_nc(totcols, specs, repeat=1, variant="full"):
    """variant: timing ablations — "full" (real kernel), "now" (skip W DMAs,
    reuse one stale tile: wrong numerics), "noout" (skip output DMAs),
    "nomm" (skip matmuls+copies: DMA streams only)."""
    import concourse.bacc as bacc
    import concourse.mybir as mybir
    import concourse.tile as tile

    f32 = mybir.dt.float32
    f32r = mybir.dt.float32r
    nc = bacc.Bacc(None, target_bir_lowering=False)
    # xt free layout: (chunk c, h*64+b) with the h=1 column block holding
    # chunk c+8 — the two packed n-halves must be contiguous because the
    # matmul stationary AP only allows one free dimension.
    xt_d = nc.declare_dram_parameter("xt", [128, NCHUNK, NH * BC], f32r,
                                     isOutput=False)
    w_d = nc.declare_dram_parameter("w", [128, totcols], f32r, isOutput=False)
    out_d = nc.declare_dram_parameter("out", [BC, NS, SIG], f32, isOutput=True)
    out_v = out_d.ap().rearrange("b s (h q j) -> s h b q j", h=NH, q=NQ, j=J)

    with tile.TileContext(nc) as tc:
        with tc.tile_pool(name="xtp", bufs=1) as xtp, \
             tc.tile_pool(name="wp", bufs=3) as wp, \
             tc.tile_pool(name="pp", bufs=8, space="PSUM") as pp, \
             tc.tile_pool(name="sp", bufs=3) as sp:
            xt = xtp.tile([128, NCHUNK, NH * BC], f32r)
            nc.sync.dma_start(xt[:], xt_d.ap())
            xt_r = xt[:]
            stale = None
            for s in [s for _ in range(repeat) for s in range(1, NS + 1)]:
                coloff, m_hi, c = specs[s - 1]
                Wd = 128 * m_hi + J
                if variant == "now" and stale is not None:
                    wt = stale
                else:
                    wt = wp.tile([128, WMAX], f32r, tag="w")
                    nc.sync.dma_start(wt[:, :Wd] if variant != "now" else wt[:],
                                      w_d.ap()[:, coloff:coloff + Wd]
                                      if variant != "now" else w_d.ap()[:, :WMAX])
                    stale = wt
                wt_r = wt[:]
                stage = sp.tile([128, NQ, J], f32, tag="stage")
                if variant != "nomm":
                    for q in range(NQ):
                        ps = pp.tile([128, J], f32)
                        for m in range(m_hi + 1):
                            ci = LPAD + 2 * q - c + m
                            stat = xt_r[:, ci, :]
                            mov = wt_r[:, 128 * (m_hi - m):128 * (m_hi - m) + J]
                            nc.tensor.matmul(ps[:], stat, mov,
                                             start=(m == 0), stop=(m == m_hi))
                        nc.vector.tensor_copy(stage[:, q, :], ps[:])
                else:
                    nc.vector.tensor_copy(stage[:, 0, 0:1], xt[:, 0, 0:1])
                if variant != "noout":
                    for h in range(NH):
                        nc.scalar.dma_start(out_v[s - 1, h],
                                            stage[64 * h:64 * h + 64, :, :])
    nc.compile()
    return nc


def _build_filters_b(kernels, scales_sqrt, trim_idx):
    """Plan B: per-scale Toeplitz stationary tiles [128, 128] with J=128,
    packed side by side. specs[s-1] = (coloff, m_hi, c)."""
    kernels = np.asarray(kernels, np.float64)
    scales_sqrt = np.asarray(scales_sqrt, np.float64)
    trim_idx = np.asarray(trim_idx)
    JB = 128
    cols = []
    specs = []
    totcols = 0
    for s in range(1, NS + 1):
        L = 16 * s + 1
        Karr = np.zeros(L + 2, np.float64)
        Karr[1:L + 1] = kernels[s - 1, :L]
        Wl = -scales_sqrt[s - 1] * (Karr[1:] - Karr[:-1])
        g = Wl[::-1].copy()
        Ks = L + 1
        start = int(trim_idx[s - 1, 0])
        SH = L - start - 1
        c = -(-SH // 128)
        m_hi = (Ks + JB - 2 - SH) // 128 + c
        # tile m: T[p, j] = g[u_m + p - j], u_m = 128*(m-c) + SH
        P = np.arange(128)[:, None]
        Jv = np.arange(JB)[None, :]
        for m in range(m_hi + 1):
            u = 128 * (m - c) + SH
            gi = u + P - Jv
            T = np.where((gi >= 0) & (gi < Ks), g[np.clip(gi, 0, Ks - 1)], 0.0)
            cols.append(T.astype(np.float32))
        specs.append((totcols, m_hi, c))
        totcols += (m_hi + 1) * JB
    return np.ascontiguousarray(np.concatenate(cols, axis=1)), specs


def _build_nc_b(totcols, specs, repeat=1, wdtype="f32r"):
    """Plan B: stationary = Toeplitz filter tile [128 taps, 128 j], moving =
    signal columns [128 taps, 8 n-spread x 64 batch], PSUM j-major.
    Output DRAM layout [s, j, g, k, b]; host transposes to [b, s, n].
    wdtype "f16": W streamed from HBM as fp16, upcast to f32r on DVE."""
    import concourse.bacc as bacc
    import concourse.mybir as mybir
    import concourse.tile as tile

    f32 = mybir.dt.float32
    f32r = mybir.dt.float32r
    f16 = mybir.dt.float16
    NCC = 11   # xt3 chunk-offset axis (cc = g - c + m + CPADB in [0, 10])
    nc = bacc.Bacc(None, target_bir_lowering=False)
    xt_d = nc.declare_dram_parameter("xt", [128, NCC, 512], f32r, isOutput=False)
    w_d = nc.declare_dram_parameter("w", [128, totcols],
                                    f16 if wdtype == "f16" else f32r,
                                    isOutput=False)
    out_d = nc.declare_dram_parameter("out", [NS, 128, 2, 8, BC], f32,
                                      isOutput=True)
    out_v = out_d.ap()

    with tile.TileContext(nc) as tc:
        with tc.tile_pool(name="xtp", bufs=1) as xtp, \
             tc.tile_pool(name="wp", bufs=3) as wp, \
             tc.tile_pool(name="wp16", bufs=3) as wp16, \
             tc.tile_pool(name="pp", bufs=6, space="PSUM") as pp, \
             tc.tile_pool(name="sp", bufs=3) as sp:
            xt = xtp.tile([128, NCC, 512], f32r)
            nc.sync.dma_start(xt[:], xt_d.ap())
            for s in [s for _ in range(repeat) for s in range(1, NS + 1)]:
                coloff, m_hi, c = specs[s - 1]
                Wd = (m_hi + 1) * 128
                wt = wp.tile([128, WMAX], f32r, tag="w")
                if wdtype == "f16":
                    wt16 = wp16.tile([128, WMAX], f16, tag="w16")
                    nc.sync.dma_start(wt16[:, :Wd],
                                      w_d.ap()[:, coloff:coloff + Wd])
                    nc.vector.tensor_copy(wt[:, :Wd], wt16[:, :Wd])
                else:
                    nc.sync.dma_start(wt[:, :Wd], w_d.ap()[:, coloff:coloff + Wd])
                stage = sp.tile([128, 2, 512], f32, tag="stage")
                for g in range(2):
                    ps = pp.tile([128, 512], f32)
                    for m in range(m_hi + 1):
                        cc = g - c + m + CPADB
                        nc.tensor.matmul(ps[:], wt[:, 128 * m:128 * m + 128],
                                         xt[:, cc, :],
                                         start=(m == 0), stop=(m == m_hi))
                    nc.vector.tensor_copy(stage[:, g, :], ps[:])
                nc.scalar.dma_start(out_v[s - 1], stage[:].rearrange(
                    "j g (k b) -> j g k b", k=8, b=BC))
    nc.compile()
    return nc


CPADB = 5


def _shard_x_b(x):
    """x -> per-core [128, 11, 512] where [p, cc, k*64+b] =
    x_pad[b, 128*(2k + cc - CPADB) + p]."""
    xs_all = np.asarray(x, np.float32).reshape(NCORES * BC, SIG)
    shards = []
    for cidx in range(NCORES):
        lin = np.zeros((NCHUNK * 128, BC), np.float32)
        lin[LPAD * 128:LPAD * 128 + SIG, :] = xs_all[cidx * BC:(cidx + 1) * BC].T
        ch = lin.reshape(NCHUNK, 128, BC)  # chunk index = pos/128 - (-LPAD)
        xt = np.zeros((128, 11, 512), np.float32)
        for cc in range(11):
            for k in range(8):
                ci = 2 * k + cc - CPADB + LPAD
                if 0 <= ci < NCHUNK:
                    xt[:, cc, k * BC:(k + 1) * BC] = ch[ci]
        shards.append(np.ascontiguousarray(xt))
    return shards


def _gather_b(per_core_outs):
    """[NS, 128, 2, 8, BC] per core -> full [B, NS, SIG].
    n = 256*k + 128*g + j."""
    outs = []
    for o in per_core_outs:
        # o[s, j, g, k, b] -> [b, s, k, g, j] -> reshape n
        t = np.ascontiguousarray(o.transpose(4, 0, 3, 2, 1))
        outs.append(t.reshape(BC, NS, SIG))
    return np.concatenate(outs, axis=0)


def _shard_x(x):
    """x [512, 1, 2048] -> list of per-core [128, NCHUNK, NH*BC] transposed
    zero-padded signal arrays. Free layout (c, h*BC+b) holds chunk c+8*h,
    so both packed n-halves sit contiguously for the matmul stationary."""
    xs_all = np.asarray(x, np.float32).reshape(NCORES * BC, SIG)
    shards = []
    for cidx in range(NCORES):
        lin = np.zeros((NCHUNK * 128, BC), np.float32)
        lin[LPAD * 128:LPAD * 128 + SIG, :] = xs_all[cidx * BC:(cidx + 1) * BC].T
        ch = lin.reshape(NCHUNK, 128, BC)
        xt = np.zeros((128, NCHUNK, NH * BC), np.float32)
        for h in range(NH):
            n = NCHUNK - 8 * h
            xt[:, :n, h * BC:(h + 1) * BC] = ch[8 * h:].transpose(1, 0, 2)
        shards.append(np.ascontiguousarray(xt))
    return shards


def _get_program(kernels, scales_sqrt, trim_idx):
    key = "prog"
    if key not in _CACHE:
        Wall, specs = _build_filters(kernels, scales_sqrt, trim_idx)
        nc = _build_nc(Wall.shape[1], specs)
        _CACHE[key] = (nc, Wall, specs)
    return _CACHE[key]


TRACE = False  # set True (e.g. from test.py) to capture a neuron profile
PLAN = "b"     # "a": batch-major PSUM; "b": j-major PSUM + host transpose


WDTYPE = "f16"  # "f32r" | "f16" — W stream precision (fp16 halves HBM traffic)


def _get_program_b(kernels, scales_sqrt, trim_idx):
    key = "prog_b" + WDTYPE
    if key not in _CACHE:
        Wall, specs = _build_filters_b(kernels, scales_sqrt, trim_idx)
        nc = _build_nc_b(Wall.shape[1], specs, wdtype=WDTYPE)
        if WDTYPE == "f16":
            Wall = Wall.astype(np.float16)
        _CACHE[key] = (nc, Wall, specs)
    return _CACHE[key]


def kernel(x, kernels, scales_sqrt, trim_idx):
    from concourse.bass_utils import run_bass_kernel_spmd

    if PLAN == "b":
        nc, Wall, _specs = _get_program_b(kernels, scales_sqrt, trim_idx)
        shards = _shard_x_b(x)
        in_maps = [{"xt": sh, "w": Wall} for sh in shards]
        res = run_bass_kernel_spmd(nc, in_maps, list(range(NCORES)), trace=TRACE)
        _CACHE["last_results"] = res
        out = _gather_b([res.results[i]["out"] for i in range(NCORES)])
        return np.ascontiguousarray(out.astype(np.float32))

    nc, Wall, _specs = _get_program(kernels, scales_sqrt, trim_idx)
    shards = _shard_x(x)
    in_maps = [{"xt": sh, "w": Wall} for sh in shards]
    res = run_bass_kernel_spmd(nc, in_maps, list(range(NCORES)), trace=TRACE)
    _CACHE["last_results"] = res
    out = np.concatenate([res.results[i]["out"] for i in range(NCORES)], axis=0)
    return np.ascontiguousarray(out.astype(np.float32))
